# revision 6
# baseline (speedup 1.0000x reference)
"""Trainium2 Bass kernel for AngularFeaturePropagation (retrieval_knn).

Computation per batch element b (one NeuronCore per b, B=8 = n_cores):
  idx[n]  = argmin_m sqrt((lt[n]-ht[m])^2 + (lp[n]-hp[m])^2)      n<8192, m<2048
  interp  = high_feats[:, idx]                                     [128, 8192]
  cat     = [low_feats; interp]                                    [192, 8192]
  y0 = W0 @ cat  -> BN(global batch stats, over all cores) -> ReLU
  y1 = W1 @ h0   -> BN -> ReLU -> out                              [128, 8192]

Device algorithm (v2):
  - scores s[n,m] = 2*lt*ht + 2*lp*hp - (ht^2+hp^2) via fp32r PE matmul
    (K=8 hi/lo split rows: fp32r is a 1+8+11-bit format at 1 cycle/row;
    the hi/lo split recovers ~22-bit mantissa accuracy, residual error
    ~1e-6 like the baseline's fp32 matmul at 4 cycles/row).
  - argmax via candidate GROUPS of 8: one DVE tensor_reduce(max) pass
    builds gmax[128 queries, 256 groups]; Max+MaxIndex run only on the
    small gmax (256 elems) instead of a second full 2048-elem pass.
  - top-2 groups per query are exported per 8-tile batch; their 64B coord
    records (8x ht + 8x hp, host-prepared table) are dma_gathered in
    token-major layout overlapping the remaining scans.
  - exact rescore of all 16 members of the top-2 groups in fp32
    ((lt-ht)^2+(lp-hp)^2, token-major): recovers the within-group offset
    and fixes any PE-rounding flips. argmin index = g*8 + offset.
  - feature gather via GPSIMD ap_gather (channel-major), chunked 4x.
  - MLP on PE; BN batch stats via bn_stats/bn_aggr + AllReduce of
    (mean_i, E[x^2]_i); biases dropped (BN shift-invariant); affine+ReLU
    fused into one ScalarE activation per layer.
"""

import sys

if '/opt/trn_rl_repo' not in sys.path:
    sys.path.insert(0, '/opt/trn_rl_repo')

import numpy as np

import concourse.bass as bass
import concourse.bacc as bacc
import concourse.tile as tile
import concourse.mybir as mybir
from concourse import bass_utils, library_config

F32 = mybir.dt.float32
F32R = mybir.dt.float32r
U16 = mybir.dt.uint16
I16 = mybir.dt.int16
U8 = mybir.dt.uint8
AF = mybir.ActivationFunctionType
OP = mybir.AluOpType
AX = mybir.AxisListType

B, N, M, C1, C2 = 8, 8192, 2048, 64, 128
NT = N // 128          # 64 query tiles
NG = M // 8            # 256 candidate groups of 8
NCH = N // 512         # 16 MLP chunks
EPS = 1e-5


def build(num_devices=8):
    nc = bacc.Bacc("TRN2", target_bir_lowering=False, debug=False, num_devices=num_devices)

    # ---- per-core DRAM I/O ----
    d_qf = nc.dram_tensor("qf", [8, N], F32R, kind="ExternalInput")
    d_cf = nc.dram_tensor("cf", [8, M], F32R, kind="ExternalInput")
    d_tbl = nc.dram_tensor("tbl", [NG, 64], F32, kind="ExternalInput")
    d_lt = nc.dram_tensor("ltt", [128, NT], F32, kind="ExternalInput")
    d_lp = nc.dram_tensor("lpt", [128, NT], F32, kind="ExternalInput")
    d_iota = nc.dram_tensor("iota16", [128, 16], F32, kind="ExternalInput")
    d_c16 = nc.dram_tensor("c16", [128, 1], F32, kind="ExternalInput")
    d_lf = nc.dram_tensor("lf", [C1, N], F32, kind="ExternalInput")
    d_hf = nc.dram_tensor("hf", [C2, M], F32, kind="ExternalInput")
    d_w0lot = nc.dram_tensor("w0lot", [C1, 128], F32, kind="ExternalInput")
    d_w0hit = nc.dram_tensor("w0hit", [C2, 128], F32, kind="ExternalInput")
    d_w1t = nc.dram_tensor("w1t", [128, 128], F32, kind="ExternalInput")
    d_g0 = nc.dram_tensor("g0", [128, 1], F32, kind="ExternalInput")
    d_be0 = nc.dram_tensor("be0", [128, 1], F32, kind="ExternalInput")
    d_g1 = nc.dram_tensor("g1", [128, 1], F32, kind="ExternalInput")
    d_be1 = nc.dram_tensor("be1", [128, 1], F32, kind="ExternalInput")
    d_out = nc.dram_tensor("out", [128, N], F32, kind="ExternalOutput")

    with tile.TileContext(nc) as tc:
        with (
            tc.tile_pool(name="persist", bufs=1) as persist,
            tc.tile_pool(name="dram", bufs=1, space="DRAM") as dram,
            tc.tile_pool(name="big", bufs=3) as bigp,
            tc.tile_pool(name="small", bufs=2) as small,
        ):
            # ---------------- stage 0: loads ----------------
            hf_sb = persist.tile([C2, M], F32)
            nc.sync.dma_start(hf_sb[:], d_hf.ap())
            nc.gpsimd.load_library(library_config.ap_gather)
            w0lot = persist.tile([C1, 128], F32)
            nc.sync.dma_start(w0lot[:], d_w0lot.ap())
            w0hit = persist.tile([C2, 128], F32)
            nc.sync.dma_start(w0hit[:], d_w0hit.ap())
            w1t = persist.tile([128, 128], F32)
            nc.sync.dma_start(w1t[:], d_w1t.ap())
            g0 = persist.tile([128, 1], F32)
            nc.sync.dma_start(g0[:], d_g0.ap())
            be0 = persist.tile([128, 1], F32)
            nc.sync.dma_start(be0[:], d_be0.ap())
            g1 = persist.tile([128, 1], F32)
            nc.sync.dma_start(g1[:], d_g1.ap())
            be1 = persist.tile([128, 1], F32)
            nc.sync.dma_start(be1[:], d_be1.ap())

            qfeat = persist.tile([8, N], F32R)
            nc.sync.dma_start(qfeat[:], d_qf.ap())
            candfeat = persist.tile([8, M], F32R)
            nc.sync.dma_start(candfeat[:], d_cf.ap())

            lt_tok = persist.tile([128, NT], F32)
            nc.sync.dma_start(lt_tok[:], d_lt.ap())
            lp_tok = persist.tile([128, NT], F32)
            nc.sync.dma_start(lp_tok[:], d_lp.ap())
            iota16 = persist.tile([128, 16], F32)
            nc.sync.dma_start(iota16[:], d_iota.ap())
            c16 = persist.tile([128, 1], F32)
            nc.sync.dma_start(c16[:], d_c16.ap())

            # ---------------- stage 1: scores + group argmax ----------------
            gidx = persist.tile([128, NT * 8], U16)  # top-8 group ids per tile
            gidxv = gidx[:].rearrange("p (t k) -> p t k", k=8)
            d_g1i = dram.tile([N], U16)
            d_g2i = dram.tile([N], U16)
            g1w = small.tile([128, N // 16], U16)
            g2w = small.tile([128, N // 16], U16)
            CK = 1024  # swdge descriptor carveout is 1024 descs
            rec1 = bigp.tile([128, NT, 64], F32, tag="big")
            rec2 = bigp.tile([128, NT, 64], F32, tag="big")
            with (
                tc.tile_pool(name="spsum", bufs=2, space="PSUM") as spsum,
                tc.tile_pool(name="gmaxp", bufs=4) as gmaxp,
                tc.tile_pool(name="sc8", bufs=4) as sc8,
            ):
                for t in range(NT):
                    ps = spsum.tile([128, M], F32)
                    for k in range(4):
                        nc.tensor.matmul(
                            ps[:, 512 * k:512 * (k + 1)],
                            qfeat[:, 128 * t:128 * (t + 1)],
                            candfeat[:, 512 * k:512 * (k + 1)],
                            start=True, stop=True,
                        )
                    gmax = gmaxp.tile([128, NG], F32)
                    nc.vector.tensor_reduce(
                        gmax[:], ps[:].rearrange("p (g k) -> p g k", k=8),
                        axis=AX.X, op=OP.max)
                    v8 = sc8.tile([128, 8], F32)
                    nc.vector.max(v8[:], gmax[:])
                    nc.vector.max_index(gidx[:, 8 * t:8 * t + 8], v8[:], gmax[:])
                    if t % 8 == 7:
                        # batch j of 8 tiles complete: export its group ids and
                        # launch its record gathers so they overlap the
                        # remaining stage-1 scans.
                        j = t // 8
                        nc.sync.dma_start(
                            d_g1i[1024 * j:1024 * (j + 1)].rearrange("(t p o) -> p t o", p=128, o=1),
                            gidxv[:, 8 * j:8 * (j + 1), 0:1])
                        nc.sync.dma_start(
                            d_g2i[1024 * j:1024 * (j + 1)].rearrange("(t p o) -> p t o", p=128, o=1),
                            gidxv[:, 8 * j:8 * (j + 1), 1:2])
                        for g in range(8):
                            nc.sync.dma_start(
                                g1w[16 * g:16 * (g + 1), 64 * j:64 * (j + 1)],
                                d_g1i[1024 * j:1024 * (j + 1)].rearrange("(s p) -> p s", p=16))
                            nc.sync.dma_start(
                                g2w[16 * g:16 * (g + 1), 64 * j:64 * (j + 1)],
                                d_g2i[1024 * j:1024 * (j + 1)].rearrange("(s p) -> p s", p=16))
                        nc.gpsimd.dma_gather(
                            rec1[:, 8 * j:8 * (j + 1), :], d_tbl.ap(),
                            g1w[:, 64 * j:64 * (j + 1)].bitcast(I16),
                            num_idxs=CK, num_idxs_reg=CK, elem_size=64,
                        )
                        nc.gpsimd.dma_gather(
                            rec2[:, 8 * j:8 * (j + 1), :], d_tbl.ap(),
                            g2w[:, 64 * j:64 * (j + 1)].bitcast(I16),
                            num_idxs=CK, num_idxs_reg=CK, elem_size=64,
                        )

            # ---------------- stage 2: exact rescore of top-2 groups ----------------
            g1t = small.tile([128, NT], U16)
            nc.sync.dma_start(g1t[:], d_g1i[:].rearrange("(t p) -> p t", p=128))
            g2t = small.tile([128, NT], U16)
            nc.sync.dma_start(g2t[:], d_g2i[:].rearrange("(t p) -> p t", p=128))
            g1f = small.tile([128, NT], F32)
            nc.vector.tensor_copy(g1f[:], g1t[:])
            g2f = small.tile([128, NT], F32)
            nc.vector.tensor_copy(g2f[:], g2t[:])

            # rec layout per rec: [128, t, 64] = [ht x8 | hp x8 | pad x48]
            lt_b = lt_tok[:].unsqueeze(2).broadcast_to((128, NT, 8))
            lp_b = lp_tok[:].unsqueeze(2).broadcast_to((128, NT, 8))
            e = small.tile([128, NT, 16], F32)
            dth = small.tile([128, NT, 8], F32, tag="dth")
            dph = small.tile([128, NT, 8], F32, tag="dph")
            for gi_, rc in ((0, rec1), (1, rec2)):
                nc.vector.tensor_tensor(dth[:], lt_b, rc[:, :, 0:8], op=OP.subtract)
                nc.vector.tensor_tensor(dph[:], lp_b, rc[:, :, 8:16], op=OP.subtract)
                nc.vector.tensor_tensor(dth[:], dth[:], dth[:], op=OP.mult)
                nc.vector.tensor_tensor(dph[:], dph[:], dph[:], op=OP.mult)
                nc.vector.tensor_tensor(e[:, :, 8 * gi_:8 * (gi_ + 1)],
                                        dth[:], dph[:], op=OP.add)

            emin = small.tile([128, NT], F32)
            nc.vector.tensor_reduce(emin[:], e[:], axis=AX.X, op=OP.min)
            mask = small.tile([128, NT, 16], U8)
            nc.vector.tensor_tensor(
                mask[:], e[:], emin[:].unsqueeze(2).broadcast_to((128, NT, 16)),
                op=OP.is_equal)
            sel = small.tile([128, NT, 16], F32)
            nc.vector.select(sel[:], mask[:],
                             iota16[:].unsqueeze(1).broadcast_to((128, NT, 16)),
                             c16[:].unsqueeze(2).broadcast_to((128, NT, 16)))
            off = small.tile([128, NT], F32)
            nc.vector.tensor_reduce(off[:], sel[:], axis=AX.X, op=OP.min)

            # idx = off < 8 ? g1*8 + off : g2*8 + off - 8
            mlt = small.tile([128, NT], U8)
            nc.vector.tensor_scalar(mlt[:], off[:], 8.0, None, op0=OP.is_lt)
            a8 = small.tile([128, NT], F32)
            nc.vector.tensor_scalar(a8[:], g1f[:], 8.0, None, op0=OP.mult)
            nc.vector.tensor_tensor(a8[:], a8[:], off[:], op=OP.add)
            b8 = small.tile([128, NT], F32)
            nc.vector.tensor_scalar(b8[:], g2f[:], 8.0, None, op0=OP.mult)
            nc.vector.tensor_tensor(b8[:], b8[:], off[:], op=OP.add)
            nc.vector.tensor_scalar(b8[:], b8[:], 8.0, None, op0=OP.subtract)
            fidx = small.tile([128, NT], F32)
            nc.vector.select(fidx[:], mlt[:], a8[:], b8[:])

            fidx_u = small.tile([128, NT], U16)
            nc.vector.tensor_copy(fidx_u[:], fidx[:])
            d_fi = dram.tile([N], U16)
            nc.sync.dma_start(d_fi[:].rearrange("(t p) -> p t", p=128), fidx_u[:])
            fiw = small.tile([128, N // 16], U16)
            for g in range(8):
                nc.sync.dma_start(fiw[16 * g:16 * (g + 1), :], d_fi[:].rearrange("(s p) -> p s", p=16))

            # ---------------- stage 3: feature gather ----------------
            interp = bigp.tile([C2, N], F32, tag="big")
            interp3 = interp[:].rearrange("p (m d) -> p m d", d=1)
            hf3 = hf_sb[:].rearrange("p (m d) -> p m d", d=1)
            for q in range(4):
                nc.gpsimd.ap_gather(
                    interp3[:, 2048 * q:2048 * (q + 1), :], hf3,
                    fiw[:, 128 * q:128 * (q + 1)].bitcast(I16),
                    channels=128, num_elems=M, d=1, num_idxs=2048,
                )

            # ---------------- stage 4/5: MLP + BN + ReLU ----------------
            d_ccin = dram.tile([128, 2], F32)
            d_ccout = dram.tile([128, 2], F32)
            d_ccin1 = dram.tile([128, 2], F32)
            d_ccout1 = dram.tile([128, 2], F32)

            def bn_apply(y_sb, st, gam, bet, d_in, d_out, out_sb, relu=True, store_to=None):
                ag = small.tile([128, 2], F32, tag="ag")
                nc.vector.bn_aggr(ag[:], st[:])
                msq = small.tile([128, 1], F32, tag="msq")
                nc.vector.tensor_mul(msq[:], ag[:, 0:1], ag[:, 0:1])
                cc = small.tile([128, 2], F32, tag="cc")
                nc.vector.tensor_copy(cc[:, 0:1], ag[:, 0:1])
                nc.vector.tensor_add(cc[:, 1:2], ag[:, 1:2], msq[:])
                nc.sync.dma_start(d_in[:], cc[:])
                if num_devices > 1:
                    nc.gpsimd.collective_compute(
                        "AllReduce", OP.add,
                        replica_groups=[list(range(num_devices))],
                        ins=[d_in[:].opt()], outs=[d_out[:].opt()],
                    )
                else:
                    nc.sync.dma_start(d_out[:], d_in[:])
                ccr = small.tile([128, 2], F32, tag="ccr")
                nc.sync.dma_start(ccr[:], d_out[:])
                mu = small.tile([128, 1], F32, tag="mu")
                nc.vector.tensor_scalar_mul(mu[:], ccr[:, 0:1], 1.0 / num_devices)
                e2g = small.tile([128, 1], F32, tag="e2g")
                nc.vector.tensor_scalar_mul(e2g[:], ccr[:, 1:2], 1.0 / num_devices)
                musq = small.tile([128, 1], F32, tag="musq")
                nc.vector.tensor_mul(musq[:], mu[:], mu[:])
                var = small.tile([128, 1], F32, tag="var")
                nc.vector.tensor_sub(var[:], e2g[:], musq[:])
                vpe = small.tile([128, 1], F32, tag="vpe")
                nc.vector.tensor_scalar_add(vpe[:], var[:], EPS)
                sd = small.tile([128, 1], F32, tag="sd")
                nc.scalar.activation(sd[:], vpe[:], AF.Sqrt)
                rs = small.tile([128, 1], F32, tag="rs")
                nc.vector.reciprocal(rs[:], sd[:])
                sc = small.tile([128, 1], F32, tag="sc")
                nc.vector.tensor_mul(sc[:], gam[:], rs[:])
                msc = small.tile([128, 1], F32, tag="msc")
                nc.vector.tensor_mul(msc[:], mu[:], sc[:])
                sh = small.tile([128, 1], F32, tag="sh")
                nc.vector.tensor_sub(sh[:], bet[:], msc[:])
                if store_to is None:
                    nc.scalar.activation(
                        out_sb[:], y_sb[:], AF.Relu if relu else AF.Copy,
                        bias=sh[:], scale=sc[:],
                    )
                else:
                    for q in range(4):
                        s_ = slice(2048 * q, 2048 * (q + 1))
                        nc.scalar.activation(
                            out_sb[:, s_], y_sb[:, s_], AF.Relu if relu else AF.Copy,
                            bias=sh[:], scale=sc[:],
                        )
                        nc.sync.dma_start(store_to[:, s_], out_sb[:, s_])

            with tc.tile_pool(name="mpsum", bufs=4, space="PSUM") as mpsum:
                # layer 0
                y0 = bigp.tile([128, N], F32, tag="big")
                st0 = persist.tile([128, NCH * 6], F32)
                lfq = []
                for q in range(4):
                    lfqt = small.tile([C1, 2048], F32, tag="lfq")
                    lfq.append(lfqt)
                    nc.sync.dma_start(lfqt[:], d_lf.ap()[:, 2048 * q:2048 * (q + 1)])
                for c in range(NCH):
                    lfch = lfq[c // 4][:, 512 * (c % 4):512 * (c % 4 + 1)]
                    ps = mpsum.tile([128, 512], F32)
                    nc.tensor.matmul(ps[:], w0lot[:], lfch,
                                     start=True, stop=False)
                    nc.tensor.matmul(ps[:], w0hit[:], interp[:, 512 * c:512 * (c + 1)],
                                     start=False, stop=True)
                    nc.vector.bn_stats(st0[:, 6 * c:6 * (c + 1)], ps[:])
                    nc.scalar.activation(y0[:, 512 * c:512 * (c + 1)], ps[:], AF.Copy)

                h0 = bigp.tile([128, N], F32, tag="big")
                bn_apply(y0, st0, g0, be0, d_ccin, d_ccout, h0)

                # layer 1
                y1 = bigp.tile([128, N], F32, tag="big")
                st1 = persist.tile([128, NCH * 6], F32)
                for c in range(NCH):
                    ps = mpsum.tile([128, 512], F32)
                    nc.tensor.matmul(ps[:], w1t[:], h0[:, 512 * c:512 * (c + 1)],
                                     start=True, stop=True)
                    nc.vector.bn_stats(st1[:, 6 * c:6 * (c + 1)], ps[:])
                    nc.scalar.activation(y1[:, 512 * c:512 * (c + 1)], ps[:], AF.Copy)

                o_sb = bigp.tile([128, N], F32, tag="big")
                bn_apply(y1, st1, g1, be1, d_ccin1, d_ccout1, o_sb,
                         store_to=d_out.ap())

    nc.compile()
    return nc


_NC_CACHE = None


def _get_nc():
    global _NC_CACHE
    if _NC_CACHE is None:
        _NC_CACHE = build()
    return _NC_CACHE


def _round_fp32r(x):
    """Round fp32 to fp32r (1+8+11 bits, RNE on bit 12)."""
    xi = np.asarray(x, np.float32).view(np.uint32).astype(np.uint64)
    lsb = (xi >> 12) & 1
    rounded = (xi + 0x7FF + lsb) & 0xFFFFF000
    return rounded.astype(np.uint32).view(np.float32)


def _split_fp32r(x):
    hi = _round_fp32r(x)
    lo = _round_fp32r(x.astype(np.float32) - hi)
    return hi, lo


def make_in_maps(inputs):
    lt = np.ascontiguousarray(inputs['low_theta'], np.float32)
    lp = np.ascontiguousarray(inputs['low_phi'], np.float32)
    lf = np.ascontiguousarray(inputs['low_feats'], np.float32)
    ht = np.ascontiguousarray(inputs['high_theta'], np.float32)
    hp = np.ascontiguousarray(inputs['high_phi'], np.float32)
    hf = np.ascontiguousarray(inputs['high_feats'], np.float32)
    W0 = np.asarray(inputs['W0'], np.float32)
    W1 = np.asarray(inputs['W1'], np.float32)
    w0lot = np.ascontiguousarray(W0[:, :C1].T)       # [64, 128]
    w0hit = np.ascontiguousarray(W0[:, C1:].T)       # [128, 128]
    w1t = np.ascontiguousarray(W1.T)                 # [128, 128]
    g0 = np.ascontiguousarray(np.asarray(inputs['g0'], np.float32).reshape(128, 1))
    be0 = np.ascontiguousarray(np.asarray(inputs['beta0'], np.float32).reshape(128, 1))
    g1 = np.ascontiguousarray(np.asarray(inputs['g1'], np.float32).reshape(128, 1))
    be1 = np.ascontiguousarray(np.asarray(inputs['beta1'], np.float32).reshape(128, 1))
    iota16 = np.ascontiguousarray(np.tile(np.arange(16, dtype=np.float32), (128, 1)))
    c16 = np.full((128, 1), 16.0, np.float32)

    in_maps = []
    for b in range(B):
        q1h, q1l = _split_fp32r(2.0 * lt[b])
        q2h, q2l = _split_fp32r(2.0 * lp[b])
        c1h, c1l = _split_fp32r(ht[b])
        c2h, c2l = _split_fp32r(hp[b])
        w = -(ht[b].astype(np.float64) ** 2 + hp[b].astype(np.float64) ** 2)
        w = w.astype(np.float32)
        wh, wl = _split_fp32r(w)
        ones = np.ones(N, np.float32)
        # row pairing: s = q1h*c1h + q1h*c1l + q1l*c1h
        #            + q2h*c2h + q2h*c2l + q2l*c2h + 1*wh + 1*wl
        qf = np.stack([q1h, q1h, q1l, q2h, q2h, q2l, ones, ones], 0)
        cf = np.stack([c1h, c1l, c1h, c2h, c2l, c2h, wh, wl], 0)
        tbl = np.zeros((NG, 64), np.float32)
        tbl[:, 0:8] = ht[b].reshape(NG, 8)
        tbl[:, 8:16] = hp[b].reshape(NG, 8)
        in_maps.append({
            "qf": np.ascontiguousarray(qf), "cf": np.ascontiguousarray(cf),
            "tbl": np.ascontiguousarray(tbl),
            "ltt": np.ascontiguousarray(lt[b].reshape(NT, 128).T),
            "lpt": np.ascontiguousarray(lp[b].reshape(NT, 128).T),
            "iota16": iota16, "c16": c16,
            "lf": lf[b], "hf": hf[b],
            "w0lot": w0lot, "w0hit": w0hit, "w1t": w1t,
            "g0": g0, "be0": be0, "g1": g1, "be1": be1,
        })
    return in_maps


def kernel(**inputs):
    nc = _get_nc()
    in_maps = make_in_maps(inputs)
    res = bass_utils.run_bass_kernel_spmd(nc, in_maps, core_ids=list(range(B)))
    out = np.stack([res.results[b]["out"] for b in range(B)], axis=0)
    return out.astype(np.float32)
